# revision 1
# baseline (speedup 1.0000x reference)
"""Chamfer distance kernel for Trainium2 (Bass/Tile), 8-core SPMD.

Problem: recon/target [64, 4, 2048] f32, mask [64, 2048] i32 ->
scalar mean chamfer loss (squared distances, masked min both directions).

Strategy (data-parallel over batch, 8 samples/core):
  - For each sample the halved negated pairwise distance matrix
        V[n, m] = x_n . y_m - (xn[n] + BIGr[n])/2 - (yn[m] + BIGc[m])/2
    is produced by ONE K=16 bf16 matmul per tile using an error-free-style
    split (x = xhi + xlo in bf16; dot = xhi.yhi + xhi.ylo + xlo.yhi, the
    dropped xlo.ylo term is ~2^-18 relative).  bf16 matmuls stream at
    1 cycle/column (fp32 is 4x slower on the PE), and bf16xbf16 products
    accumulate exactly in fp32 PSUM, so this is fp32-grade accuracy at 4x
    the speed.  Norm rows are hi/lo split the same way.  Row-side vectors
    carry +BIG*(1-mask)/2 (invalid rows -> V=+BIG/2 -> relu(-2*max)=0: no
    mask multiply needed), column-side vectors carry -BIG*(1-mask)/2
    (invalid columns excluded from the max).  max_m V = -d2min/2, recovered
    exactly by relu(-2*max) in the epilogue (the clamp commutes with min).
  - Per 128-row block the PE fills PSUM [128, 2048] as two [128,1024] tiles;
    ScalarE stages the second half to SBUF; one VectorE MAX2_REDUCE custom-DVE
    op (authored here: out = max(in0,in1), accum_out = max-reduce) absorbs
    both halves at 2 elem/lane/cycle and emits the row max directly.  Both
    chamfer directions run as separate matmul orientations (x-rows / y-rows).
  - Four samples pack per 128-partition operand tensor at 32-partition slots
    (matmul lhsT base-partition constraint), with explicit tile_position.
  - Epilogue: relu(-2*max) on ScalarE, partition sum via ones-matmul, block
    sums via a 3D-AP reduce.  Output per core: sums [2, 8] + cnt [8, 1]; the
    masked means and batch mean happen on host.
"""

import sys

import numpy as np

for _p in ("/opt/trn_rl_repo",):
    if _p not in sys.path:
        sys.path.append(_p)

B, F, N = 64, 4, 2048
N_CORES = 8
SPC = B // N_CORES  # samples per core
NB = N // 128  # 128-row blocks per sample
BIGV = 1.0e30
NEG_INIT = -3.0e38

_CACHE = {}


def _register_max2_reduce():
    """Author + register a custom DVE op: out = max(in0, in1),
    accum_out = max-reduce(out) seeded from s0.  Absorbs two tiles per pass
    (one read port each) with the row-max fused — the core absorption
    primitive of this kernel."""
    from concourse import dve_ops
    from concourse.dve_spec import Spec, Src0, Src1, C0, maxx, lower, _has_src1
    from concourse.dve_uop import DveOpSpec

    NAME = "MAX2_REDUCE_ANT"
    for op in dve_ops.OPS:
        if op.name == NAME:
            return op

    def _ref_max2(in0, in1, c0, c1, c2):
        b = np.maximum(in0.astype(np.float32), in1.astype(np.float32))
        a = np.maximum(b.reshape(b.shape[0], -1).max(axis=-1, keepdims=True), c0)
        return b, a

    spec = Spec(body=maxx(Src0, Src1), accum=maxx, accum_init=C0,
                reference=_ref_max2)
    row = dve_ops._CUSTOM_DVE_ROW_BASE + len(dve_ops.OPS)
    shas = {}
    for ver in ("v3", "v4"):
        s = DveOpSpec(name=NAME, opcode=row, uops=lower(spec, ver=ver),
                      rd1_en=_has_src1(spec))
        shas[ver] = s.sha(ver)
    op = dve_ops.DveOp(NAME, spec, subdim=False, uops_sha=shas)
    dve_ops.OPS.append(op)
    dve_ops._SUB_OPCODE_FOR_NAME[NAME] = row
    dve_ops.CUSTOM_DVE_SPECS[NAME] = spec
    return op


def _build_bass():
    from contextlib import ExitStack

    import concourse.mybir as mybir
    import concourse.tile as tile
    from concourse import bacc

    max2 = _register_max2_reduce()

    f32 = mybir.dt.float32
    bf16 = mybir.dt.bfloat16
    Alu = mybir.AluOpType
    Act = mybir.ActivationFunctionType
    Axis = mybir.AxisListType

    nc = bacc.Bacc("TRN2", target_bir_lowering=False, debug=False,
                   num_devices=N_CORES)

    recon = nc.dram_tensor("recon", (SPC, F, N), f32, kind="ExternalInput").ap()
    target = nc.dram_tensor("target", (SPC, F, N), f32, kind="ExternalInput").ap()
    maskf = nc.dram_tensor("maskf", (SPC, N), f32, kind="ExternalInput").ap()
    sums_out = nc.dram_tensor("sums", (2, SPC), f32, kind="ExternalOutput").ap()
    cnt_out = nc.dram_tensor("cnt", (SPC, 1), f32, kind="ExternalOutput").ap()

    with tile.TileContext(nc) as tc, ExitStack() as ctx:
        # ---- persistent pools ----
        consts = ctx.enter_context(tc.tile_pool(name="consts", bufs=1))
        opnds = ctx.enter_context(tc.tile_pool(name="opnds", bufs=1))
        accum = ctx.enter_context(tc.tile_pool(name="accum", bufs=1))

        ones_col = consts.tile([128, 1], f32)
        nc.gpsimd.memset(ones_col, 1.0)
        ones2 = consts.tile([2, N], bf16)
        nc.gpsimd.memset(ones2, 1.0)
        # negE64 [64, 8]: -0.5 on the (4-row) block diagonal, replicated at
        # partition 0 (for x) and partition 32 (for y)
        negE = consts.tile([2 * SPC * F, SPC], f32, name="negE")
        nc.gpsimd.memset(negE, -0.5)
        for base in (0, 32):
            nc.gpsimd.affine_select(out=negE[base:base + 32, :],
                                    in_=negE[base:base + 32, :],
                                    compare_op=Alu.is_ge, fill=0.0,
                                    base=0, pattern=[[-F, SPC]],
                                    channel_multiplier=1)
            nc.gpsimd.affine_select(out=negE[base:base + 32, :],
                                    in_=negE[base:base + 32, :],
                                    compare_op=Alu.is_ge, fill=0.0,
                                    base=F - 1, pattern=[[F, SPC]],
                                    channel_multiplier=-1)

        m_sb = opnds.tile([SPC, N], f32)
        nc.sync.dma_start(out=m_sb, in_=maskf)

        # operand tensors (bf16): [orientation][group]; sample slot s lives at
        # partitions [32s, 32s+16):
        #   lhsT rows: 0-3 xhi | 4-7 xhi | 8-11 xlo | 12 rvh | 13 rvl | 14-15 1
        #   rhs  rows: 0-3 yhi | 4-7 ylo | 8-11 yhi | 12-13 1 | 14 cvh | 15 cvl
        lhsT_t = [[opnds.tile([128, N], bf16, tag=f"L{o}{g}", name=f"L{o}{g}")
                   for g in range(2)] for o in range(2)]
        rhs_t = [[opnds.tile([128, N], bf16, tag=f"R{o}{g}", name=f"R{o}{g}")
                  for g in range(2)] for o in range(2)]
        negmax = [accum.tile([128, 128], f32, tag=f"nm{o}", name=f"nm{o}")
                  for o in range(2)]

        # ---- prep: hi/lo splits, norms, masked norm vectors, assembly ----
        # prep_a holds the large f32 staging (freed before the main loop so
        # the stage pool reuses ONLY this early-released memory); prep_b holds
        # the bf16 split products consumed by the assembly DMAs.
        with tc.tile_pool(name="prep_a", bufs=1) as prep_a, \
                tc.tile_pool(name="prep_b", bufs=1) as prep_b, \
                tc.tile_pool(name="prep_ps", bufs=1, space="PSUM") as prep_ps:
            # x at partitions 0-31, y at partitions 32-63
            xy = prep_a.tile([2 * SPC * F, N], f32, tag="xy")
            nc.sync.dma_start(out=xy[:SPC * F, :],
                              in_=recon.rearrange("b f n -> (b f) n"))
            nc.sync.dma_start(out=xy[SPC * F:, :],
                              in_=target.rearrange("b f n -> (b f) n"))

            def hilo(src, tag, rows=128):
                """bf16 hi/lo split: hi = bf16(src), lo = bf16(src - hi).
                The f32 diff scratch shares one slot across all splits."""
                p = src.shape[0]
                hi = prep_b.tile([p, N], bf16, tag=f"{tag}_h", name=f"{tag}_h")
                df = prep_a.tile([128, N], f32, tag="hilo_d", name=f"{tag}_d")
                lo = prep_b.tile([p, N], bf16, tag=f"{tag}_l", name=f"{tag}_l")
                nc.scalar.copy(hi, src)
                nc.vector.tensor_sub(df[:p], src, hi)
                nc.scalar.copy(lo, df[:p])
                return hi, lo

            xyh, xyl = hilo(xy, "xy")

            sq = prep_a.tile([2 * SPC * F, N], f32, tag="sq")
            nc.scalar.square(sq[:SPC * F, :], xy[:SPC * F, :])
            nc.scalar.square(sq[SPC * F:, :], xy[SPC * F:, :])

            # -xn/2, -yn/2 via block-diagonal -(1/2) ones matmuls (K=32, M=8)
            ps_xn = prep_ps.tile([SPC, N], f32, tag="psxn")
            ps_yn = prep_ps.tile([SPC, N], f32, tag="psyn")
            for c in range(N // 512):
                sl = slice(c * 512, (c + 1) * 512)
                nc.tensor.matmul(ps_xn[:, sl], negE[0:32, :], sq[0:32, sl],
                                 start=True, stop=True, tile_position=(0, 0))
                nc.tensor.matmul(ps_yn[:, sl], negE[32:64, :], sq[32:64, sl],
                                 start=True, stop=True, tile_position=(32, 0))

            # all four masked norm vectors in one tensor (32-aligned slots):
            # rows 0-7 xr | 32-39 xc | 64-71 yr | 96-103 yc
            nf = prep_a.tile([128, N], f32, tag="nf")
            nc.gpsimd.memset(nf, 0.0)

            # BIG masks (halved): bp = +BIG*(1-m)/2, bn = -BIG*(1-m)/2
            bp = prep_a.tile([SPC, N], f32, tag="bp")
            bn = prep_a.tile([SPC, N], f32, tag="bn")
            nc.vector.tensor_scalar(out=bp, in0=m_sb, scalar1=-1.0,
                                    scalar2=-BIGV / 2, op0=Alu.add,
                                    op1=Alu.mult)
            nc.vector.tensor_scalar(out=bn, in0=m_sb, scalar1=-1.0,
                                    scalar2=BIGV / 2, op0=Alu.add,
                                    op1=Alu.mult)
            nc.vector.tensor_add(nf[0:SPC, :], ps_xn, bp)
            nc.vector.tensor_add(nf[32:32 + SPC, :], ps_xn, bn)
            nc.vector.tensor_add(nf[64:64 + SPC, :], ps_yn, bp)
            nc.vector.tensor_add(nf[96:96 + SPC, :], ps_yn, bn)
            nfh, nfl = hilo(nf, "nf")

            # assembly: per-slot row DMAs (plain 2D APs)
            for o in range(2):
                dlo = 0 if o == 0 else 32         # lhsT data rows in xyh/xyl
                dro = 32 if o == 0 else 0         # rhs data rows
                rvo = 0 if o == 0 else 64         # row-vector base in nfh/nfl
                cvo = 96 if o == 0 else 32        # col-vector base
                for g in range(2):
                    L = lhsT_t[o][g]
                    R = rhs_t[o][g]
                    for s in range(4):
                        j = g * 4 + s
                        p0 = 32 * s
                        dl = slice(dlo + 4 * j, dlo + 4 * j + 4)
                        dr = slice(dro + 4 * j, dro + 4 * j + 4)
                        rv = slice(rvo + j, rvo + j + 1)
                        cv = slice(cvo + j, cvo + j + 1)
                        nc.sync.dma_start(out=L[p0:p0 + 4, :], in_=xyh[dl])
                        nc.sync.dma_start(out=L[p0 + 4:p0 + 8, :], in_=xyh[dl])
                        nc.sync.dma_start(out=L[p0 + 8:p0 + 12, :], in_=xyl[dl])
                        nc.sync.dma_start(out=L[p0 + 12:p0 + 13, :], in_=nfh[rv])
                        nc.sync.dma_start(out=L[p0 + 13:p0 + 14, :], in_=nfl[rv])
                        nc.sync.dma_start(out=L[p0 + 14:p0 + 16, :], in_=ones2)
                        nc.sync.dma_start(out=R[p0:p0 + 4, :], in_=xyh[dr])
                        nc.sync.dma_start(out=R[p0 + 4:p0 + 8, :], in_=xyl[dr])
                        nc.sync.dma_start(out=R[p0 + 8:p0 + 12, :], in_=xyh[dr])
                        nc.sync.dma_start(out=R[p0 + 12:p0 + 14, :], in_=ones2)
                        nc.sync.dma_start(out=R[p0 + 14:p0 + 15, :], in_=nfh[cv])
                        nc.sync.dma_start(out=R[p0 + 15:p0 + 16, :], in_=nfl[cv])

        # ---- main loop ----
        with tc.tile_pool(name="stage", bufs=4) as stage, \
                tc.tile_pool(name="mm_ps", bufs=2, space="PSUM") as mm_ps:
            for o in range(2):
                for g in range(2):
                    for s in range(4):
                        j = g * 4 + s
                        p0 = 32 * s
                        L = lhsT_t[o][g]
                        R = rhs_t[o][g]
                        for i in range(NB):
                            lhs = L[p0:p0 + 16, i * 128:(i + 1) * 128]
                            ph0 = mm_ps.tile([128, 1024], f32, tag="ph0")
                            ph1 = mm_ps.tile([128, 1024], f32, tag="ph1")
                            for c in range(2):
                                nc.tensor.matmul(
                                    ph0[:, c * 512:(c + 1) * 512], lhs,
                                    R[p0:p0 + 16, c * 512:(c + 1) * 512],
                                    start=True, stop=True,
                                    tile_position=(p0, 0))
                            for c in range(2):
                                nc.tensor.matmul(
                                    ph1[:, c * 512:(c + 1) * 512], lhs,
                                    R[p0:p0 + 16, 1024 + c * 512:1024 + (c + 1) * 512],
                                    start=True, stop=True,
                                    tile_position=(p0, 0))
                            staged = stage.tile([128, 1024], f32, tag="staged")
                            nc.scalar.copy(staged, ph1)
                            mout = stage.tile([128, 1024], f32, tag="mout")
                            nc.vector._custom_dve(
                                max2, out=mout, in0=ph0, in1=staged,
                                s0=NEG_INIT,
                                accum_out=negmax[o][:, j * NB + i:j * NB + i + 1])

        # ---- epilogue ----
        with tc.tile_pool(name="ep", bufs=1) as ep, \
                tc.tile_pool(name="ep_ps", bufs=1, space="PSUM") as ep_ps:
            for o in range(2):
                relu_t = ep.tile([128, 128], f32, tag=f"relu{o}",
                                 name=f"relu{o}")
                nc.scalar.activation(relu_t, negmax[o], Act.Relu,
                                     bias=0.0, scale=-2.0)
                ps = ep_ps.tile([1, 128], f32, tag=f"eps{o}", name=f"eps{o}")
                nc.tensor.matmul(ps, ones_col, relu_t, start=True, stop=True)
                s_sb = ep.tile([1, SPC], f32, tag=f"ssb{o}", name=f"ssb{o}")
                nc.vector.tensor_reduce(
                    s_sb, ps.rearrange("p (s i) -> p s i", s=SPC),
                    Axis.X, Alu.add)
                nc.sync.dma_start(out=sums_out[o:o + 1, :], in_=s_sb)
            cnt_sb = ep.tile([SPC, 1], f32, tag="cnt")
            nc.vector.tensor_reduce(cnt_sb, m_sb, Axis.X, Alu.add)
            nc.sync.dma_start(out=cnt_out, in_=cnt_sb)

    nc.compile()
    return nc


def kernel(recon, target, mask):
    if "nc" not in _CACHE:
        _CACHE["nc"] = _build_bass()
    nc = _CACHE["nc"]
    from concourse.bass_utils import run_bass_kernel_spmd

    recon = np.ascontiguousarray(recon, dtype=np.float32)
    target = np.ascontiguousarray(target, dtype=np.float32)
    maskf = np.ascontiguousarray(mask.astype(np.float32))

    in_maps = []
    for c in range(N_CORES):
        sl = slice(c * SPC, (c + 1) * SPC)
        in_maps.append({
            "recon": np.ascontiguousarray(recon[sl]),
            "target": np.ascontiguousarray(target[sl]),
            "maskf": np.ascontiguousarray(maskf[sl]),
        })

    res = run_bass_kernel_spmd(nc, in_maps, core_ids=list(range(N_CORES)))

    loss_sum = 0.0
    for r in res.results:
        s = r["sums"].astype(np.float64)
        cnt = r["cnt"].astype(np.float64).ravel()
        loss_sum += float(np.sum((s[0] + s[1]) / cnt))
    loss = loss_sum / B
    return np.array(loss, dtype=np.float32)



# revision 5
# speedup vs baseline: 1.9297x; 1.9297x over previous
"""Chamfer distance kernel for Trainium2 (Bass/Tile), 8-core SPMD.

Problem: recon/target [64, 4, 2048] f32, mask [64, 2048] i32 ->
scalar mean chamfer loss (squared distances, masked min both directions).

Strategy (data-parallel over batch, 8 samples/core), v2: mask compaction.

The mask keeps ~50% of the 2048 points.  The host compacts each sample's
valid points to the front and pads to PAD=1152 columns with a far-point
sentinel (16,16,16,16) whose squared distance (>=~480) can never win the
min against any valid point, so padded columns need no BIG masking and
padded rows are simply dropped on the host.  This shrinks each sample's
pairwise matrix from 2048^2 to 1152^2 (~3.2x less device work).

Device program per core (8 samples, 2 chamfer orientations):
  - Host pre-assembles bf16 operand tensors (error-free hi/lo split:
    dot = xhi.yhi + xhi.ylo + xlo.yhi, dropped xlo.ylo ~ 2^-16) with the
    halved negated column-norm vector -(yn)/2 riding as two extra K rows
    (ones x cvh/cvl), K=14 per 32-partition sample slot:
        V[n, m] = x_n . y_m - yn[m]/2     (= (xn[n] - d2[n,m]) / 2 ... )
    so rowmax_m V = (xn[n] - d2min[n])/2 and the host recovers
    d2min = xn - 2*rowmax (clamped at 0) -- the per-row norm is applied
    post-hoc on the host, which keeps K small and needs no BIG terms.
  - Per 128-row block: 3 matmuls fill PSUM [128, 1152] (<=512-col chunks);
    ScalarE stages cols [576:1152) to SBUF; one VectorE
    tensor_tensor_reduce absorbs both halves (out = max(in0,in1),
    accum = row-max seeded at -3e38) and emits the complete row max of
    the block directly into negmax[o][:, j*NB+i].
  - Epilogue: DMA the two [128, 8*NB] negmax tiles to HBM; the host does
    relu/masking/means in numpy (O(B*N), negligible).
"""

import sys

import numpy as np

for _p in ("/opt/trn_rl_repo",):
    if _p not in sys.path:
        sys.path.append(_p)

B, F, N = 64, 4, 2048
N_CORES = 8
SPC = B // N_CORES  # samples per core
FAR = 16.0          # far-point sentinel coordinate
NEG_INIT = -3.0e38

_CACHE = {}


def _register_max2_reduce():
    """Author + register a custom DVE op: out = max(in0, in1),
    accum_out = max-reduce(out) seeded from s0.  Absorbs two tiles per pass
    with the row-max fused."""
    from concourse import dve_ops
    from concourse.dve_spec import Spec, Src0, Src1, C0, maxx, lower, _has_src1
    from concourse.dve_uop import DveOpSpec

    NAME = "MAX2_REDUCE_ANT"
    for op in dve_ops.OPS:
        if op.name == NAME:
            return op

    def _ref_max2(in0, in1, c0, c1, c2):
        b = np.maximum(in0.astype(np.float32), in1.astype(np.float32))
        a = np.maximum(b.reshape(b.shape[0], -1).max(axis=-1, keepdims=True), c0)
        return b, a

    spec = Spec(body=maxx(Src0, Src1), accum=maxx, accum_init=C0,
                reference=_ref_max2)
    row = dve_ops._CUSTOM_DVE_ROW_BASE + len(dve_ops.OPS)
    shas = {}
    for ver in ("v3", "v4"):
        s = DveOpSpec(name=NAME, opcode=row, uops=lower(spec, ver=ver),
                      rd1_en=_has_src1(spec))
        shas[ver] = s.sha(ver)
    op = dve_ops.DveOp(NAME, spec, subdim=False, uops_sha=shas)
    dve_ops.OPS.append(op)
    dve_ops._SUB_OPCODE_FOR_NAME[NAME] = row
    dve_ops.CUSTOM_DVE_SPECS[NAME] = spec
    return op


def _build_bass(pad):
    """Build the per-core program for padded point count `pad` (mult of 128)."""
    from contextlib import ExitStack

    import concourse.mybir as mybir
    import concourse.tile as tile
    from concourse import bacc

    f32 = mybir.dt.float32
    bf16 = mybir.dt.bfloat16

    max2 = _register_max2_reduce()

    nb = pad // 128
    half = pad // 2
    # matmul output chunks (PSUM accumulation groups are <= 512 f32)
    chunks = []
    c = 0
    while c < pad:
        chunks.append((c, min(c + 512, pad)))
        c += 512

    nc = bacc.Bacc("TRN2", target_bir_lowering=False, debug=False,
                   num_devices=N_CORES)

    L_dram = [[nc.dram_tensor(f"L{o}{g}", (128, pad), bf16,
                              kind="ExternalInput").ap()
               for g in range(2)] for o in range(2)]
    R_dram = [[nc.dram_tensor(f"R{o}{g}", (128, pad), bf16,
                              kind="ExternalInput").ap()
               for g in range(2)] for o in range(2)]
    nm_out = [nc.dram_tensor(f"nm{o}", (128, SPC * nb), f32,
                             kind="ExternalOutput").ap() for o in range(2)]

    with tile.TileContext(nc) as tc, ExitStack() as ctx:
        opnds = ctx.enter_context(tc.tile_pool(name="opnds", bufs=1))
        accum = ctx.enter_context(tc.tile_pool(name="accum", bufs=1))

        L_sb = [[opnds.tile([128, pad], bf16, tag=f"L{o}{g}", name=f"L{o}{g}")
                 for g in range(2)] for o in range(2)]
        R_sb = [[opnds.tile([128, pad], bf16, tag=f"R{o}{g}", name=f"R{o}{g}")
                 for g in range(2)] for o in range(2)]
        negmax = [accum.tile([128, SPC * nb], f32, tag=f"nm{o}", name=f"nm{o}")
                  for o in range(2)]

        # operand loads, ordered so the first-used group arrives first
        for o in range(2):
            for g in range(2):
                nc.sync.dma_start(out=L_sb[o][g], in_=L_dram[o][g])
                nc.sync.dma_start(out=R_sb[o][g], in_=R_dram[o][g])

        with tc.tile_pool(name="stage", bufs=4) as stage, \
                tc.tile_pool(name="mm_ps", bufs=2, space="PSUM") as mm_ps:
            for o in range(2):
                for g in range(2):
                    for s in range(4):
                        j = g * 4 + s
                        p0 = 32 * s
                        Lt = L_sb[o][g]
                        Rt = R_sb[o][g]
                        for i in range(nb):
                            lhs = Lt[p0:p0 + 14, i * 128:(i + 1) * 128]
                            ps = mm_ps.tile([128, pad], f32, tag="ps")
                            for (c0, c1) in chunks:
                                nc.tensor.matmul(
                                    ps[:, c0:c1], lhs, Rt[p0:p0 + 14, c0:c1],
                                    start=True, stop=True,
                                    tile_position=(p0, 0))
                            st = stage.tile([128, half], f32, tag="st")
                            nc.scalar.copy(st, ps[:, half:])
                            junk = stage.tile([128, half], f32, tag="junk")
                            nc.vector._custom_dve(
                                max2, out=junk, in0=ps[:, :half], in1=st,
                                s0=NEG_INIT,
                                accum_out=negmax[o][:, j * nb + i:
                                                    j * nb + i + 1])

        with tc.tile_pool(name="ep", bufs=1):
            for o in range(2):
                nc.sync.dma_start(out=nm_out[o], in_=negmax[o])

    nc.compile()
    return nc


def _get_nc(pad):
    key = ("nc", pad)
    if key not in _CACHE:
        _CACHE[key] = _build_bass(pad)
    return _CACHE[key]


def _prep_core(recon_c, target_c, mask_c, pad):
    """Build the four operand tensors for one core + host-side leftovers.

    Returns (in_map, post) where post holds what the host needs to finish:
    per sample j: cnt, xn[:cnt], yn[:cnt].
    """
    import ml_dtypes

    bf16 = ml_dtypes.bfloat16
    L = [[np.zeros((128, pad), dtype=bf16) for _ in range(2)] for _ in range(2)]
    R = [[np.zeros((128, pad), dtype=bf16) for _ in range(2)] for _ in range(2)]
    post = []

    for j in range(SPC):
        m = mask_c[j] != 0
        cnt = int(m.sum())
        xp = np.full((F, pad), FAR, dtype=np.float32)
        yp = np.full((F, pad), FAR, dtype=np.float32)
        xp[:, :cnt] = recon_c[j][:, m]
        yp[:, :cnt] = target_c[j][:, m]
        xn = np.sum(xp * xp, axis=0)  # [pad]
        yn = np.sum(yp * yp, axis=0)

        xh = xp.astype(bf16)
        xl = (xp - xh.astype(np.float32)).astype(bf16)
        yh = yp.astype(bf16)
        yl = (yp - yh.astype(np.float32)).astype(bf16)
        cvy = (-0.5 * yn).astype(np.float32)
        cvyh = cvy.astype(bf16)
        cvyl = (cvy - cvyh.astype(np.float32)).astype(bf16)
        cvx = (-0.5 * xn).astype(np.float32)
        cvxh = cvx.astype(bf16)
        cvxl = (cvx - cvxh.astype(np.float32)).astype(bf16)

        g, s = j // 4, j % 4
        p0 = 32 * s
        one = np.ones((pad,), dtype=bf16)
        for o in range(2):
            dh, dl = (xh, xl) if o == 0 else (yh, yl)      # lhsT data
            rh, rl = (yh, yl) if o == 0 else (xh, xl)      # rhs data
            ch, cl = (cvyh, cvyl) if o == 0 else (cvxh, cvxl)
            Lt, Rt = L[o][g], R[o][g]
            Lt[p0 + 0:p0 + 4] = dh
            Lt[p0 + 4:p0 + 8] = dh
            Lt[p0 + 8:p0 + 12] = dl
            Lt[p0 + 12] = one
            Lt[p0 + 13] = one
            Rt[p0 + 0:p0 + 4] = rh
            Rt[p0 + 4:p0 + 8] = rl
            Rt[p0 + 8:p0 + 12] = rh
            Rt[p0 + 12] = ch
            Rt[p0 + 13] = cl
        post.append((cnt, xn[:cnt].astype(np.float64),
                     yn[:cnt].astype(np.float64)))

    in_map = {}
    for o in range(2):
        for g in range(2):
            in_map[f"L{o}{g}"] = L[o][g]
            in_map[f"R{o}{g}"] = R[o][g]
    return in_map, post


def kernel(recon, target, mask):
    recon = np.ascontiguousarray(recon, dtype=np.float32)
    target = np.ascontiguousarray(target, dtype=np.float32)
    mask_b = np.asarray(mask) != 0

    cnts = mask_b.sum(axis=1)
    pad = 1152 if int(cnts.max()) <= 1152 else 2048
    nb = pad // 128
    nc = _get_nc(pad)

    from concourse.bass_utils import run_bass_kernel_spmd

    in_maps = []
    posts = []
    for c in range(N_CORES):
        sl = slice(c * SPC, (c + 1) * SPC)
        im, post = _prep_core(recon[sl], target[sl], mask_b[sl], pad)
        in_maps.append(im)
        posts.append(post)

    res = run_bass_kernel_spmd(nc, in_maps, core_ids=list(range(N_CORES)))

    loss_sum = 0.0
    for c in range(N_CORES):
        nm = [np.asarray(res.results[c][f"nm{o}"], dtype=np.float64)
              for o in range(2)]
        for j in range(SPC):
            cnt, xn, yn = posts[c][j]
            per = 0.0
            for o in range(2):
                vs = nm[o][:, j * nb:(j + 1) * nb].T.reshape(-1)  # [pad]
                norms = xn if o == 0 else yn
                d2 = norms - 2.0 * vs[:cnt]
                per += float(np.maximum(d2, 0.0).sum()) / cnt
            loss_sum += per
    loss = loss_sum / B
    return np.array(loss, dtype=np.float32)


# revision 8
# speedup vs baseline: 2.0229x; 1.0483x over previous
"""Chamfer distance kernel for Trainium2 (Bass/Tile), 8-core SPMD.

Problem: recon/target [64, 4, 2048] f32, mask [64, 2048] i32 ->
scalar mean chamfer loss (squared distances, masked min both directions).

Strategy (data-parallel over batch, 8 samples/core), v2: mask compaction.

The mask keeps ~50% of the 2048 points.  The host compacts each sample's
valid points to the front and pads to PAD=1152 columns with a far-point
sentinel (16,16,16,16) whose squared distance (>=~480) can never win the
min against any valid point, so padded columns need no BIG masking and
padded rows are simply dropped on the host.  This shrinks each sample's
pairwise matrix from 2048^2 to 1152^2 (~3.2x less device work).

Device program per core (8 samples, 2 chamfer orientations):
  - Host pre-assembles bf16 operand tensors (error-free hi/lo split:
    dot = xhi.yhi + xhi.ylo + xlo.yhi, dropped xlo.ylo ~ 2^-16) with the
    halved negated column-norm vector -(yn)/2 riding as two extra K rows
    (ones x cvh/cvl), K=14 per 32-partition sample slot:
        V[n, m] = x_n . y_m - yn[m]/2     (= (xn[n] - d2[n,m]) / 2 ... )
    so rowmax_m V = (xn[n] - d2min[n])/2 and the host recovers
    d2min = xn - 2*rowmax (clamped at 0) -- the per-row norm is applied
    post-hoc on the host, which keeps K small and needs no BIG terms.
  - Per 128-row block: 3 matmuls fill PSUM [128, 1152] (<=512-col chunks);
    ScalarE stages cols [576:1152) to SBUF; one VectorE
    tensor_tensor_reduce absorbs both halves (out = max(in0,in1),
    accum = row-max seeded at -3e38) and emits the complete row max of
    the block directly into negmax[o][:, j*NB+i].
  - Epilogue: DMA the two [128, 8*NB] negmax tiles to HBM; the host does
    relu/masking/means in numpy (O(B*N), negligible).
"""

import sys

import numpy as np

for _p in ("/opt/trn_rl_repo",):
    if _p not in sys.path:
        sys.path.append(_p)

B, F, N = 64, 4, 2048
N_CORES = 8
SPC = B // N_CORES  # samples per core
FAR = 16.0          # far-point sentinel coordinate
NEG_INIT = -3.0e38

_CACHE = {}


def _register_max2_reduce():
    """Author + register a custom DVE op: out = max(in0, in1),
    accum_out = max-reduce(out) seeded from s0.  Absorbs two tiles per pass
    with the row-max fused."""
    from concourse import dve_ops
    from concourse.dve_spec import Spec, Src0, Src1, C0, maxx, lower, _has_src1
    from concourse.dve_uop import DveOpSpec

    NAME = "MAX2_REDUCE_ANT"
    for op in dve_ops.OPS:
        if op.name == NAME:
            return op

    def _ref_max2(in0, in1, c0, c1, c2):
        b = np.maximum(in0.astype(np.float32), in1.astype(np.float32))
        a = np.maximum(b.reshape(b.shape[0], -1).max(axis=-1, keepdims=True), c0)
        return b, a

    spec = Spec(body=maxx(Src0, Src1), accum=maxx, accum_init=C0,
                reference=_ref_max2)
    row = dve_ops._CUSTOM_DVE_ROW_BASE + len(dve_ops.OPS)
    shas = {}
    for ver in ("v3", "v4"):
        s = DveOpSpec(name=NAME, opcode=row, uops=lower(spec, ver=ver),
                      rd1_en=_has_src1(spec))
        shas[ver] = s.sha(ver)
    op = dve_ops.DveOp(NAME, spec, subdim=False, uops_sha=shas)
    dve_ops.OPS.append(op)
    dve_ops._SUB_OPCODE_FOR_NAME[NAME] = row
    dve_ops.CUSTOM_DVE_SPECS[NAME] = spec
    return op


def _build_bass(pad):
    """Build the per-core program for padded point count `pad` (mult of 128)."""
    from contextlib import ExitStack

    import concourse.mybir as mybir
    import concourse.tile as tile
    from concourse import bacc

    f32 = mybir.dt.float32
    bf16 = mybir.dt.bfloat16

    max2 = _register_max2_reduce()

    nb = pad // 128
    half = pad // 2
    # matmul output chunks (PSUM accumulation groups are <= 512 f32).
    # Ordered so the staged half [half:pad) is produced FIRST: the ScalarE
    # stage copy can then start while the PE still fills the fold half.
    chunks = []
    c = 0
    while c < pad:
        chunks.append((c, min(c + 512, pad)))
        c += 512
    chunks.sort(key=lambda ch: ch[0] < half)

    nc = bacc.Bacc("TRN2", target_bir_lowering=False, debug=False,
                   num_devices=N_CORES)

    L_dram = [[nc.dram_tensor(f"L{o}{g}", (128, pad), bf16,
                              kind="ExternalInput").ap()
               for g in range(2)] for o in range(2)]
    R_dram = [[nc.dram_tensor(f"R{o}{g}", (128, pad), bf16,
                              kind="ExternalInput").ap()
               for g in range(2)] for o in range(2)]
    nm_out = [nc.dram_tensor(f"nm{o}", (128, SPC * nb), f32,
                             kind="ExternalOutput").ap() for o in range(2)]

    with tile.TileContext(nc) as tc, ExitStack() as ctx:
        opnds = ctx.enter_context(tc.tile_pool(name="opnds", bufs=1))
        accum = ctx.enter_context(tc.tile_pool(name="accum", bufs=1))

        L_sb = [[opnds.tile([128, pad], bf16, tag=f"L{o}{g}", name=f"L{o}{g}")
                 for g in range(2)] for o in range(2)]
        R_sb = [[opnds.tile([128, pad], bf16, tag=f"R{o}{g}", name=f"R{o}{g}")
                 for g in range(2)] for o in range(2)]
        negmax = [accum.tile([128, SPC * nb], f32, tag=f"nm{o}", name=f"nm{o}")
                  for o in range(2)]

        # operand loads, ordered so the first-used group arrives first
        for o in range(2):
            for g in range(2):
                nc.sync.dma_start(out=L_sb[o][g], in_=L_dram[o][g])
                nc.sync.dma_start(out=R_sb[o][g], in_=R_dram[o][g])

        with tc.tile_pool(name="stage", bufs=6) as stage, \
                tc.tile_pool(name="mm_ps", bufs=2, space="PSUM") as mm_ps:
            for o in range(2):
                for g in range(2):
                    for s in range(4):
                        j = g * 4 + s
                        p0 = 32 * s
                        Lt = L_sb[o][g]
                        Rt = R_sb[o][g]
                        for i in range(nb):
                            lhs = Lt[p0:p0 + 14, i * 128:(i + 1) * 128]
                            ps = mm_ps.tile([128, pad], f32, tag="ps")
                            for (c0, c1) in chunks:
                                nc.tensor.matmul(
                                    ps[:, c0:c1], lhs, Rt[p0:p0 + 14, c0:c1],
                                    start=True, stop=True,
                                    tile_position=(p0, 0))
                            st = stage.tile([128, half], f32, tag="st")
                            nc.scalar.copy(st, ps[:, half:])
                            junk = stage.tile([128, half], f32, tag="junk")
                            nc.vector._custom_dve(
                                max2, out=junk, in0=ps[:, :half], in1=st,
                                s0=NEG_INIT,
                                accum_out=negmax[o][:, j * nb + i:
                                                    j * nb + i + 1])

        with tc.tile_pool(name="ep", bufs=1):
            for o in range(2):
                nc.sync.dma_start(out=nm_out[o], in_=negmax[o])

    nc.compile()
    return nc


def _get_nc(pad):
    key = ("nc", pad)
    if key not in _CACHE:
        _CACHE[key] = _build_bass(pad)
    return _CACHE[key]


def _prep_core(recon_c, target_c, mask_c, pad):
    """Build the four operand tensors for one core + host-side leftovers.

    Returns (in_map, post) where post holds what the host needs to finish:
    per sample j: cnt, xn[:cnt], yn[:cnt].
    """
    import ml_dtypes

    bf16 = ml_dtypes.bfloat16
    L = [[np.zeros((128, pad), dtype=bf16) for _ in range(2)] for _ in range(2)]
    R = [[np.zeros((128, pad), dtype=bf16) for _ in range(2)] for _ in range(2)]
    post = []

    for j in range(SPC):
        m = mask_c[j] != 0
        cnt = int(m.sum())
        xp = np.full((F, pad), FAR, dtype=np.float32)
        yp = np.full((F, pad), FAR, dtype=np.float32)
        xp[:, :cnt] = recon_c[j][:, m]
        yp[:, :cnt] = target_c[j][:, m]
        xn = np.sum(xp * xp, axis=0)  # [pad]
        yn = np.sum(yp * yp, axis=0)

        xh = xp.astype(bf16)
        xl = (xp - xh.astype(np.float32)).astype(bf16)
        yh = yp.astype(bf16)
        yl = (yp - yh.astype(np.float32)).astype(bf16)
        cvy = (-0.5 * yn).astype(np.float32)
        cvyh = cvy.astype(bf16)
        cvyl = (cvy - cvyh.astype(np.float32)).astype(bf16)
        cvx = (-0.5 * xn).astype(np.float32)
        cvxh = cvx.astype(bf16)
        cvxl = (cvx - cvxh.astype(np.float32)).astype(bf16)

        g, s = j // 4, j % 4
        p0 = 32 * s
        one = np.ones((pad,), dtype=bf16)
        for o in range(2):
            dh, dl = (xh, xl) if o == 0 else (yh, yl)      # lhsT data
            rh, rl = (yh, yl) if o == 0 else (xh, xl)      # rhs data
            ch, cl = (cvyh, cvyl) if o == 0 else (cvxh, cvxl)
            Lt, Rt = L[o][g], R[o][g]
            Lt[p0 + 0:p0 + 4] = dh
            Lt[p0 + 4:p0 + 8] = dh
            Lt[p0 + 8:p0 + 12] = dl
            Lt[p0 + 12] = one
            Lt[p0 + 13] = one
            Rt[p0 + 0:p0 + 4] = rh
            Rt[p0 + 4:p0 + 8] = rl
            Rt[p0 + 8:p0 + 12] = rh
            Rt[p0 + 12] = ch
            Rt[p0 + 13] = cl
        post.append((cnt, xn[:cnt].astype(np.float64),
                     yn[:cnt].astype(np.float64)))

    in_map = {}
    for o in range(2):
        for g in range(2):
            in_map[f"L{o}{g}"] = L[o][g]
            in_map[f"R{o}{g}"] = R[o][g]
    return in_map, post


def kernel(recon, target, mask):
    recon = np.ascontiguousarray(recon, dtype=np.float32)
    target = np.ascontiguousarray(target, dtype=np.float32)
    mask_b = np.asarray(mask) != 0

    cnts = mask_b.sum(axis=1)
    pad = 1152 if int(cnts.max()) <= 1152 else 2048
    nb = pad // 128
    nc = _get_nc(pad)

    from concourse.bass_utils import run_bass_kernel_spmd

    in_maps = []
    posts = []
    for c in range(N_CORES):
        sl = slice(c * SPC, (c + 1) * SPC)
        im, post = _prep_core(recon[sl], target[sl], mask_b[sl], pad)
        in_maps.append(im)
        posts.append(post)

    res = run_bass_kernel_spmd(nc, in_maps, core_ids=list(range(N_CORES)))

    loss_sum = 0.0
    for c in range(N_CORES):
        nm = [np.asarray(res.results[c][f"nm{o}"], dtype=np.float64)
              for o in range(2)]
        for j in range(SPC):
            cnt, xn, yn = posts[c][j]
            per = 0.0
            for o in range(2):
                vs = nm[o][:, j * nb:(j + 1) * nb].T.reshape(-1)  # [pad]
                norms = xn if o == 0 else yn
                d2 = norms - 2.0 * vs[:cnt]
                per += float(np.maximum(d2, 0.0).sum()) / cnt
            loss_sum += per
    loss = loss_sum / B
    return np.array(loss, dtype=np.float32)
